# revision 39
# baseline (speedup 1.0000x reference)
"""Trainium2 Bass kernel for the MoE-routed 3-layer LoRA MLP.

Strategy: pure data-parallel over the batch (16384 rows -> 2048 per core,
8 cores, no collectives). On-device layout is feature-major (transposed):
activations live as [features, batch] so every matmul contracts over the
partition dimension without any on-device transposes. All matmul operands
are bf16 (PSUM accumulation is f32).

The domain-routing network is x-independent and tiny (M=8 domains, L=3
layers, E=4 experts), so it is constant-folded on the host into a
per-token, per-layer gamma row-vector (zeta * alpha expanded over expert
rank, replicated x4 for the PE row-band trick) and DMA'd in directly.

Per core the three layers are fused column-by-column (4 columns of 512
tokens): weights for all layers stay SBUF-resident; h1/h2 never touch
DRAM. x columns are prefetched one column ahead; the main-path PSUM pool
holds 5 banks so o-group drains never stall the accumulation stream.
"""

import json

import numpy as np
import ml_dtypes

import concourse.bass as bass
import concourse.tile as tile
from concourse import mybir
from concourse.bass_utils import run_bass_kernel_spmd

F32 = mybir.dt.float32
BF16 = mybir.dt.bfloat16
AF = mybir.ActivationFunctionType
ALU = mybir.AluOpType
AX = mybir.AxisListType

N_CORES = 8
BSZ, D0, D1, D2, D3 = 16384, 2048, 2048, 1024, 512
E, RK, M, H, L = 4, 8, 8, 64, 3
B_LOC = BSZ // N_CORES  # 2048
NT = 4                  # batch columns per core
NB = B_LOC // NT        # 512
BF_NP = ml_dtypes.bfloat16

N_WARM_INIT = 56        # PE ramp while the first DMAs land
N_WARM_PER_A = 0        # fillers between DMA-paced A-side matmuls (col 0 only)
N_WARM_OG0_WAIT = 20    # fillers while col 0 waits for the first W1 chunk


# ---------------------------------------------------------------------------
# BIR post-pass: this container's walrus rejects instructions carrying more
# than one semaphore wait; split extras onto preceding same-engine NoOps
# (the engine sequencer processes waits before the instruction, so this is
# semantics-preserving).
# ---------------------------------------------------------------------------
def _split_waits(bir, max_waits=1):
    counter = [0]

    def fix_block(bb):
        new_instructions = []
        for ins in bb.get("instructions", []):
            si = ins.get("sync_info") or {}
            waits = si.get("on_wait") or []
            if len(waits) > max_waits:
                head, tail = waits[:-max_waits], waits[-max_waits:]
                for i in range(0, len(head), max_waits):
                    counter[0] += 1
                    new_instructions.append(
                        {
                            "engine": ins["engine"],
                            "ins": [],
                            "name": f"I-waitsplit-{counter[0]}",
                            "opcode": "Drain",
                            "outs": [],
                            "sync_info": {
                                "on_update": [],
                                "on_wait": head[i : i + max_waits],
                            },
                        }
                    )
                si = dict(si)
                si["on_wait"] = tail
                ins = dict(ins)
                ins["sync_info"] = si
            new_instructions.append(ins)
        if "instructions" in bb:
            bb["instructions"] = new_instructions
        for inner in bb.get("blocks", []):
            fix_block(inner)

    for fn in bir.get("functions", []):
        for bb in fn.get("blocks", []):
            fix_block(bb)
    return bir


def _patch_bass_json(nc):
    orig = nc.to_json_bytes

    def wrapped(*a, **k):
        return json.dumps(_split_waits(json.loads(orig(*a, **k)))).encode()

    nc.to_json_bytes = wrapped


# ---------------------------------------------------------------------------
# Full per-core graph
# ---------------------------------------------------------------------------
def _build(nc):
    DIMS = [(D0, D1), (D1, D2), (D2, D3)]

    xt = nc.dram_tensor("xt", [D0, B_LOC], BF16, kind="ExternalInput")
    combine_d = nc.dram_tensor("combine", [128, 128], BF16, kind="ExternalInput")
    wts = [
        nc.dram_tensor(f"w{l + 1}t", [i, o], BF16, kind="ExternalInput")
        for l, (i, o) in enumerate(DIMS)
    ]
    ats = [
        nc.dram_tensor(f"a{l + 1}t", [i, E * RK], BF16, kind="ExternalInput")
        for l, (i, _) in enumerate(DIMS)
    ]
    lbs = [
        nc.dram_tensor(f"lb{l + 1}", [128, o], BF16, kind="ExternalInput")
        for l, (_, o) in enumerate(DIMS)
    ]
    biases = [
        nc.dram_tensor(f"bias{l + 1}", [o], F32, kind="ExternalInput")
        for l, (_, o) in enumerate(DIMS)
    ]
    gds = [
        nc.dram_tensor(f"g{l + 1}", [128, B_LOC], BF16, kind="ExternalInput")
        for l in range(L)
    ]
    out_d = nc.dram_tensor("out", [D3, B_LOC], BF16, kind="ExternalOutput")

    with tile.TileContext(nc) as tc:
        with (
            tc.tile_pool(name="const", bufs=1) as const,
            tc.tile_pool(name="wpool", bufs=1) as wpool,
            tc.tile_pool(name="gpool", bufs=1) as gpool,
            tc.tile_pool(name="dram", bufs=1, space="DRAM") as dram,
            tc.tile_pool(name="xcol", bufs=2) as xcolp,
            tc.tile_pool(name="h1", bufs=16) as h1p,
            tc.tile_pool(name="h2", bufs=8) as h2p,
            tc.tile_pool(name="oc", bufs=2) as ocp,
            tc.tile_pool(name="tw", bufs=4) as twp,
            tc.tile_pool(name="mmps", bufs=5, space="PSUM") as mmps,
            tc.tile_pool(name="warmp", bufs=1, space="PSUM") as warmp,
            tc.tile_pool(name="tps", bufs=2, space="PSUM") as tps,
        ):
            # --- PE warmup: ramp the p-state while the first DMAs land ------
            warm_src = const.tile([128, 128], BF16, tag="warm_src")
            nc.vector.memset(warm_src[:], 0.0)
            warm_sink = dram.tile([128, 128], BF16, tag="warm_sink")
            warm_ps = warmp.tile([128, 128], F32, tag="warm", name="warm_ps")

            def warmup(count):
                for _ in range(count):
                    nc.tensor.matmul(warm_ps[:], warm_src[:], warm_src[:],
                                     start=True, stop=True)

            warmup(N_WARM_INIT)

            # per-layer per-token gamma rows (host-computed); g1 goes on the
            # fast gpsimd ring (needed ~16us in), g2/g3 on the scalar ring
            gammas = []
            for l in range(L):
                g = gpool.tile([128, B_LOC], BF16, tag=f"g{l}", name=f"gamma{l}")
                gammas.append(g)
            combine_t = const.tile([128, 128], BF16, tag="combine")
            nc.sync.dma_start(out=combine_t[:], in_=combine_d[:])

            # resident weights: A/lb/bias per layer + W (single tile per layer)
            a_tiles = [[] for _ in range(L)]
            w_views = [None] * L
            lb_tiles = [None] * L
            b_tiles = [None] * L

            def load_a(l):
                IN, OUT = DIMS[l]
                KT = IN // 128
                a_all = wpool.tile([128, KT * E * RK], BF16, tag=f"a{l}", name=f"a{l}_all")
                a_src = ats[l][:]
                a_src = bass.AP(
                    tensor=a_src.tensor, offset=a_src.offset,
                    ap=[[E * RK, 128], [128 * E * RK, KT], [1, E * RK]],
                )
                nc.gpsimd.dma_start(out=a_all.rearrange("p (kt er) -> p kt er", kt=KT), in_=a_src)
                for k in range(KT):
                    a_tiles[l].append(a_all[:, k * E * RK : (k + 1) * E * RK])

            def load_lb(l):
                IN, OUT = DIMS[l]
                lb_tiles[l] = wpool.tile([128, OUT], BF16, tag=f"lb{l}", name=f"lb{l}")
                nc.gpsimd.dma_start(out=lb_tiles[l][:], in_=lbs[l][:])
                b_tiles[l] = wpool.tile([128, OUT // 128], F32, tag=f"b{l}", name=f"b{l}")
                nc.gpsimd.dma_start(
                    out=b_tiles[l][:], in_=biases[l].rearrange("(o p) -> p o", p=128)
                )

            def load_layer(l, nchunk, chunks=None):
                # one fanned SWDGE transfer per chunk (~770 GB/s aggregate)
                IN, OUT = DIMS[l]
                KT = IN // 128
                if w_views[l] is None:
                    w_all = wpool.tile([128, KT * OUT], BF16, tag=f"w{l}", name=f"w{l}_all")
                    w_views[l] = w_all.rearrange("p (kt o) -> p kt o", kt=KT)
                wv = w_views[l]
                src = wts[l][:]
                cw = OUT // nchunk
                for c in chunks if chunks is not None else range(nchunk):
                    chunk_src = bass.AP(
                        tensor=src.tensor, offset=src.offset + c * cw,
                        ap=[[OUT, 128], [128 * OUT, KT], [1, cw]],
                    )
                    nc.gpsimd.dma_start(out=wv[:, :, c * cw : (c + 1) * cw], in_=chunk_src)

            def alloc_xcol(n):
                KT = D0 // 128
                xk = xcolp.tile([128, KT * NB], BF16, tag="xcol", name=f"x{n}")
                xv = xk.rearrange("p (kt b) -> p kt b", kt=KT)
                return xv

            def emit_xchunk(xv, n, c, nchunk):
                KT = D0 // 128
                ck = KT // nchunk
                src = xt[:]
                chunk_src = bass.AP(
                    tensor=src.tensor,
                    offset=src.offset + n * NB + c * ck * 128 * B_LOC,
                    ap=[[B_LOC, 128], [128 * B_LOC, ck], [1, NB]],
                )
                nc.gpsimd.dma_start(out=xv[:, c * ck : (c + 1) * ck, :], in_=chunk_src)

            def load_xcol(n):
                xv = alloc_xcol(n)
                emit_xchunk(xv, n, 0, 1)
                return [xv[:, k, :] for k in range(D0 // 128)]

            # gpsimd queue ordered by first use, x col0 and W1 chunks
            # interleaved so the first o-group can start ~16us in
            load_a(0)
            xv0 = alloc_xcol(0)
            emit_xchunk(xv0, 0, 0, 4)
            load_layer(0, nchunk=8, chunks=[0])
            emit_xchunk(xv0, 0, 1, 4)
            load_layer(0, nchunk=8, chunks=[1])
            emit_xchunk(xv0, 0, 2, 4)
            load_layer(0, nchunk=8, chunks=[2])
            emit_xchunk(xv0, 0, 3, 4)
            load_layer(0, nchunk=8, chunks=[3])
            load_lb(0)
            nc.gpsimd.dma_start(out=gammas[0][:], in_=gds[0][:])
            load_layer(0, nchunk=8, chunks=[4, 5, 6, 7])
            first_cols = [xv0[:, k, :] for k in range(D0 // 128)]
            load_a(1)
            load_layer(1, nchunk=1)
            load_lb(1)
            nc.gpsimd.dma_start(out=gammas[1][:], in_=gds[1][:])
            load_a(2)
            load_layer(2, nchunk=1)
            load_lb(2)
            nc.gpsimd.dma_start(out=gammas[2][:], in_=gds[2][:])

            def emit_A(l, n, cols, part, k0, k1, KT):
                """LoRA A-side, col-group packed: 4 concurrent partial
                accumulations in one PSUM bank (the four 32-row bands run on
                disjoint PE column groups). Emitted in two ranges: the last
                band group (which depends on the previous layer's final relu)
                is deferred under the first main o-group chain."""
                for k in range(k0, k1):
                    g = k % 4
                    nc.tensor.matmul(
                        part[g * 32 : (g + 1) * 32, :], a_tiles[l][k], cols[k],
                        start=(k < 4), stop=(k >= KT - 4), tile_position=(0, g * 32),
                    )
                    if n == 0 and l == 0 and k % 4 == 3 and k < KT - 1:
                        warmup(10)  # x col-0 chunks arrive ~2.5us apart

            # main fused pipeline: per batch-column, all three layers
            xcols = [first_cols, None, None, None]
            for n in range(NT):
                cols = xcols[n]
                if n + 1 < NT:
                    xcols[n + 1] = load_xcol(n + 1)
                for l, (IN, OUT) in enumerate(DIMS):
                    KT, OT = IN // 128, OUT // 128
                    part = tps.tile([128, NB], F32, tag="tpsum")
                    emit_A(l, n, cols, part, 0, KT - 4, KT)
                    if n == 0 and l == 0:
                        warmup(N_WARM_OG0_WAIT)
                    tw = twp.tile([128, NB], BF16, tag="tw")
                    pt = None

                    nxt = []
                    if n == 0 and l == 0:
                        # small first groups: start once 1MB of W1 has landed
                        ogroups = [(0, 2), (2, 2), (4, 4), (8, 4), (12, 4)]
                    else:
                        ogroups = [(og, min(4, OT - og)) for og in range(0, OT, 4)]
                    for og, gw in ogroups:
                        pss = []
                        for i in range(gw):
                            o = og + i
                            ps = mmps.tile([128, NB], F32, tag="mm")
                            for k in range(KT):
                                nc.tensor.matmul(
                                    ps[:], w_views[l][:, k, o * 128 : (o + 1) * 128],
                                    cols[k], start=(k == 0), stop=False,
                                )
                            if og == 0 and i == 0:
                                # deferred A band group (waits the previous
                                # layer's last relu) + partials copy-out
                                emit_A(l, n, cols, part, KT - 4, KT, KT)
                                pt = twp.tile([128, NB], BF16, tag="tw",
                                              name=f"pt{l}_{n}")
                                nc.vector.tensor_copy(pt[:], part[:])
                            elif og == 0 and i == 1:
                                # combine + gamma-mul hidden under this chain
                                t4 = tps.tile([128, NB], F32, tag="tpsum")
                                nc.tensor.matmul(t4[:], combine_t[:], pt[:],
                                                 start=True, stop=True)
                                nc.vector.tensor_mul(
                                    tw[:], t4[:], gammas[l][:, n * NB : (n + 1) * NB]
                                )
                            pss.append(ps)
                        if l == 2 and og == 0:
                            oc_big = ocp.tile([128, OT * NB], BF16, tag="oc", name=f"oc{n}")
                            ocv = oc_big.rearrange("p (ot b) -> p ot b", ot=OT)
                        for i in range(gw):
                            o = og + i
                            nc.tensor.matmul(
                                pss[i][:],
                                lb_tiles[l][i * 32 : (i + 1) * 32, o * 128 : (o + 1) * 128],
                                tw[i * 32 : (i + 1) * 32, :],
                                start=False, stop=True, tile_position=(i * 32, 0),
                            )
                            if l < 2:
                                pool = h1p if l == 0 else h2p
                                ot = pool.tile([128, NB], BF16, tag=f"h{l + 1}", name=f"h{l}_{n}_{o}")
                                dst = ot[:]
                            else:
                                dst = ocv[:, o, :]
                            if i % 2 == 0:
                                nc.scalar.activation(
                                    dst, pss[i][:], AF.Relu, bias=b_tiles[l][:, o : o + 1]
                                )
                            else:
                                nc.vector.tensor_scalar(
                                    dst, pss[i][:], b_tiles[l][:, o : o + 1], 0.0,
                                    op0=ALU.add, op1=ALU.max,
                                )
                            if l < 2:
                                nxt.append(dst)
                        if l == 2:
                            # last column: two half-transfers so the first can
                            # overlap the remaining relus
                            nhalf = 2 if n == NT - 1 else 1
                            oth = OT // nhalf
                            for hf in range(nhalf):
                                out_dst = bass.AP(
                                    tensor=out_d[:].tensor,
                                    offset=out_d[:].offset + n * NB + hf * oth * 128 * B_LOC,
                                    ap=[[B_LOC, 128], [128 * B_LOC, oth], [1, NB]],
                                )
                                ring = nc.sync if (nhalf == 2 and hf == 1) else nc.gpsimd
                                ring.dma_start(
                                    out=out_dst, in_=ocv[:, hf * oth : (hf + 1) * oth, :]
                                )
                    cols = nxt
                if n == 0:
                    # drain the warmup accumulator mid-run, off the tail
                    wout = const.tile([128, 128], BF16, tag="warm_out", name="warmout")
                    nc.vector.tensor_copy(wout[:], warm_ps[:])
                    nc.scalar.dma_start(out=warm_sink[:], in_=wout[:])
    return nc


_CACHED = {}


def _get_nc():
    if "nc" not in _CACHED:
        nc = bass.Bass()
        _build(nc)
        _patch_bass_json(nc)
        _CACHED["nc"] = nc
    return _CACHED["nc"]


# ---------------------------------------------------------------------------
# Host-side routing: x-independent, tiny -> fold into per-token gamma rows
# ---------------------------------------------------------------------------
def _topk_sparse_softmax_np(logits, k):
    n = logits.shape[-1]
    flat = logits.reshape(-1, n)
    thresh = np.sort(flat, axis=-1)[:, n - k][:, None]
    mask = flat >= thresh
    mx = np.max(flat, axis=-1, keepdims=True)
    e = np.exp(flat - mx) * mask
    return (e / e.sum(-1, keepdims=True)).reshape(logits.shape)


def _routing_host(inputs):
    f = lambda k: np.asarray(inputs[k], np.float64)
    dom_emb, layer_pos = f("dom_emb"), f("layer_pos")
    router_inp = np.concatenate(
        [
            np.broadcast_to(dom_emb[:, None, :], (M, L, H)),
            np.broadcast_to(layer_pos[None, :, :], (M, L, H)),
        ],
        axis=-1,
    )                                                       # [M,L,2H]
    hz = np.maximum(router_inp @ f("Wi1").T + f("bi1"), 0.0)
    zeta_logits = (hz @ f("Wi2").T + f("bi2"))[..., 0]      # [M,L]
    zeta_all = _topk_sparse_softmax_np(zeta_logits, 2)
    ha = np.maximum(router_inp @ f("Wa1").T + f("ba1"), 0.0)
    alpha_logits = ha @ f("Wa2").T + f("ba2")               # [M,L,E]
    alpha_all = _topk_sparse_softmax_np(alpha_logits, 2)
    g = f("gate_logits")
    R = np.log1p(np.exp(-np.abs(g))) + np.maximum(g, 0.0)   # stable softplus
    R = R * f("R_benefit")
    R = R / np.clip(R.sum(-1, keepdims=True), 1e-12, None)
    zeta_agg = R @ zeta_all                                 # [M,L]
    alpha_agg = np.einsum("mn,nle->mle", R, alpha_all)      # [M,L,E]
    return (zeta_agg[:, :, None] * alpha_agg).astype(np.float32)  # [M,L,E]


def kernel(**inputs) -> np.ndarray:
    x = np.asarray(inputs["x"], np.float32)
    ids = np.asarray(inputs["domain_ids"]).astype(np.int64)
    f32 = lambda a: np.ascontiguousarray(np.asarray(a), np.float32)
    bf = lambda a: np.ascontiguousarray(np.asarray(a, np.float32).astype(BF_NP))

    W = [f32(inputs[f"W{i}"]) for i in (1, 2, 3)]
    Bv = [f32(inputs[f"b{i}"]) for i in (1, 2, 3)]
    A = [f32(inputs[f"A{i}"]) for i in (1, 2, 3)]
    Bl = [f32(inputs[f"B{i}"]) for i in (1, 2, 3)]

    gamma = _routing_host(inputs)                           # [M,L,E]
    gtok = gamma[ids]                                       # [B,L,E]

    shared = {"combine": bf(np.tile(np.eye(E * RK, dtype=np.float32), (4, 4)))}
    for l in range(3):
        shared[f"w{l + 1}t"] = bf(W[l].T)
        shared[f"a{l + 1}t"] = bf(A[l].reshape(E * RK, -1).T)
        shared[f"lb{l + 1}"] = bf(np.tile(Bl[l].transpose(0, 2, 1).reshape(E * RK, -1), (4, 1)))
        shared[f"bias{l + 1}"] = Bv[l]

    in_maps = []
    for i in range(N_CORES):
        sl = slice(i * B_LOC, (i + 1) * B_LOC)
        m = dict(shared)
        m["xt"] = bf(x[sl].T)
        for l in range(3):
            g32 = np.repeat(gtok[sl, l, :], RK, axis=1).T   # [32, B_LOC]
            m[f"g{l + 1}"] = bf(np.tile(g32, (4, 1)))       # [128, B_LOC]
        in_maps.append(m)

    nc = _get_nc()
    res = run_bass_kernel_spmd(nc, in_maps, core_ids=list(range(N_CORES)))
    return np.concatenate(
        [np.asarray(res.results[i]["out"]).astype(np.float32).T for i in range(N_CORES)],
        axis=0,
    )


# revision 40
# speedup vs baseline: 1.0014x; 1.0014x over previous
"""Trainium2 Bass kernel for the MoE-routed 3-layer LoRA MLP.

Strategy: pure data-parallel over the batch (16384 rows -> 2048 per core,
8 cores, no collectives). On-device layout is feature-major (transposed):
activations live as [features, batch] so every matmul contracts over the
partition dimension without any on-device transposes. All matmul operands
are bf16 (PSUM accumulation is f32).

The domain-routing network is x-independent and tiny (M=8 domains, L=3
layers, E=4 experts), so it is constant-folded on the host into a
per-token, per-layer gamma row-vector (zeta * alpha expanded over expert
rank, replicated x4 for the PE row-band trick) and DMA'd in directly.

Per core the three layers are fused column-by-column (4 columns of 512
tokens): weights for all layers stay SBUF-resident; h1/h2 never touch
DRAM. x columns are prefetched one column ahead; the main-path PSUM pool
holds 5 banks so o-group drains never stall the accumulation stream.
"""

import json

import numpy as np
import ml_dtypes

import concourse.bass as bass
import concourse.tile as tile
from concourse import mybir
from concourse.bass_utils import run_bass_kernel_spmd

F32 = mybir.dt.float32
BF16 = mybir.dt.bfloat16
AF = mybir.ActivationFunctionType
ALU = mybir.AluOpType
AX = mybir.AxisListType

N_CORES = 8
BSZ, D0, D1, D2, D3 = 16384, 2048, 2048, 1024, 512
E, RK, M, H, L = 4, 8, 8, 64, 3
B_LOC = BSZ // N_CORES  # 2048
NT = 4                  # batch columns per core
NB = B_LOC // NT        # 512
BF_NP = ml_dtypes.bfloat16

N_WARM_INIT = 56        # PE ramp while the first DMAs land
N_WARM_PER_A = 0        # fillers between DMA-paced A-side matmuls (col 0 only)
N_WARM_OG0_WAIT = 20    # fillers while col 0 waits for the first W1 chunk


# ---------------------------------------------------------------------------
# BIR post-pass: this container's walrus rejects instructions carrying more
# than one semaphore wait; split extras onto preceding same-engine NoOps
# (the engine sequencer processes waits before the instruction, so this is
# semantics-preserving).
# ---------------------------------------------------------------------------
def _split_waits(bir, max_waits=1):
    counter = [0]

    def fix_block(bb):
        new_instructions = []
        for ins in bb.get("instructions", []):
            si = ins.get("sync_info") or {}
            waits = si.get("on_wait") or []
            if len(waits) > max_waits:
                head, tail = waits[:-max_waits], waits[-max_waits:]
                for i in range(0, len(head), max_waits):
                    counter[0] += 1
                    new_instructions.append(
                        {
                            "engine": ins["engine"],
                            "ins": [],
                            "name": f"I-waitsplit-{counter[0]}",
                            "opcode": "Drain",
                            "outs": [],
                            "sync_info": {
                                "on_update": [],
                                "on_wait": head[i : i + max_waits],
                            },
                        }
                    )
                si = dict(si)
                si["on_wait"] = tail
                ins = dict(ins)
                ins["sync_info"] = si
            new_instructions.append(ins)
        if "instructions" in bb:
            bb["instructions"] = new_instructions
        for inner in bb.get("blocks", []):
            fix_block(inner)

    for fn in bir.get("functions", []):
        for bb in fn.get("blocks", []):
            fix_block(bb)
    return bir


def _patch_bass_json(nc):
    orig = nc.to_json_bytes

    def wrapped(*a, **k):
        return json.dumps(_split_waits(json.loads(orig(*a, **k)))).encode()

    nc.to_json_bytes = wrapped


# ---------------------------------------------------------------------------
# Full per-core graph
# ---------------------------------------------------------------------------
def _build(nc):
    DIMS = [(D0, D1), (D1, D2), (D2, D3)]

    xt = nc.dram_tensor("xt", [D0, B_LOC], BF16, kind="ExternalInput")
    combine_d = nc.dram_tensor("combine", [128, 128], BF16, kind="ExternalInput")
    wts = [
        nc.dram_tensor(f"w{l + 1}t", [i, o], BF16, kind="ExternalInput")
        for l, (i, o) in enumerate(DIMS)
    ]
    ats = [
        nc.dram_tensor(f"a{l + 1}t", [i, E * RK], BF16, kind="ExternalInput")
        for l, (i, _) in enumerate(DIMS)
    ]
    lbs = [
        nc.dram_tensor(f"lb{l + 1}", [128, o], BF16, kind="ExternalInput")
        for l, (_, o) in enumerate(DIMS)
    ]
    biases = [
        nc.dram_tensor(f"bias{l + 1}", [o], F32, kind="ExternalInput")
        for l, (_, o) in enumerate(DIMS)
    ]
    gds = [
        nc.dram_tensor(f"g{l + 1}", [128, B_LOC], BF16, kind="ExternalInput")
        for l in range(L)
    ]
    out_d = nc.dram_tensor("out", [D3, B_LOC], BF16, kind="ExternalOutput")

    with tile.TileContext(nc) as tc:
        with (
            tc.tile_pool(name="const", bufs=1) as const,
            tc.tile_pool(name="wpool", bufs=1) as wpool,
            tc.tile_pool(name="gpool", bufs=1) as gpool,
            tc.tile_pool(name="dram", bufs=1, space="DRAM") as dram,
            tc.tile_pool(name="xcol", bufs=2) as xcolp,
            tc.tile_pool(name="h1", bufs=16) as h1p,
            tc.tile_pool(name="h2", bufs=8) as h2p,
            tc.tile_pool(name="oc", bufs=2) as ocp,
            tc.tile_pool(name="tw", bufs=4) as twp,
            tc.tile_pool(name="mmps", bufs=5, space="PSUM") as mmps,
            tc.tile_pool(name="warmp", bufs=1, space="PSUM") as warmp,
            tc.tile_pool(name="tps", bufs=2, space="PSUM") as tps,
        ):
            # --- PE warmup: ramp the p-state while the first DMAs land ------
            warm_src = const.tile([128, 128], BF16, tag="warm_src")
            nc.vector.memset(warm_src[:], 0.0)
            warm_sink = dram.tile([128, 128], BF16, tag="warm_sink")
            warm_ps = warmp.tile([128, 128], F32, tag="warm", name="warm_ps")

            def warmup(count):
                for _ in range(count):
                    nc.tensor.matmul(warm_ps[:], warm_src[:], warm_src[:],
                                     start=True, stop=True)

            warmup(N_WARM_INIT)

            # per-layer per-token gamma rows (host-computed); g1 goes on the
            # fast gpsimd ring (needed ~16us in), g2/g3 on the scalar ring
            gammas = []
            for l in range(L):
                g = gpool.tile([128, B_LOC], BF16, tag=f"g{l}", name=f"gamma{l}")
                gammas.append(g)
            combine_t = const.tile([128, 128], BF16, tag="combine")
            nc.sync.dma_start(out=combine_t[:], in_=combine_d[:])

            # resident weights: A/lb/bias per layer + W (single tile per layer)
            a_tiles = [[] for _ in range(L)]
            w_views = [None] * L
            lb_tiles = [None] * L
            b_tiles = [None] * L

            def load_a(l):
                IN, OUT = DIMS[l]
                KT = IN // 128
                a_all = wpool.tile([128, KT * E * RK], BF16, tag=f"a{l}", name=f"a{l}_all")
                a_src = ats[l][:]
                a_src = bass.AP(
                    tensor=a_src.tensor, offset=a_src.offset,
                    ap=[[E * RK, 128], [128 * E * RK, KT], [1, E * RK]],
                )
                nc.gpsimd.dma_start(out=a_all.rearrange("p (kt er) -> p kt er", kt=KT), in_=a_src)
                for k in range(KT):
                    a_tiles[l].append(a_all[:, k * E * RK : (k + 1) * E * RK])

            def load_lb(l):
                IN, OUT = DIMS[l]
                lb_tiles[l] = wpool.tile([128, OUT], BF16, tag=f"lb{l}", name=f"lb{l}")
                nc.gpsimd.dma_start(out=lb_tiles[l][:], in_=lbs[l][:])
                b_tiles[l] = wpool.tile([128, OUT // 128], F32, tag=f"b{l}", name=f"b{l}")
                nc.gpsimd.dma_start(
                    out=b_tiles[l][:], in_=biases[l].rearrange("(o p) -> p o", p=128)
                )

            def load_layer(l, nchunk, chunks=None):
                # one fanned SWDGE transfer per chunk (~770 GB/s aggregate)
                IN, OUT = DIMS[l]
                KT = IN // 128
                if w_views[l] is None:
                    w_all = wpool.tile([128, KT * OUT], BF16, tag=f"w{l}", name=f"w{l}_all")
                    w_views[l] = w_all.rearrange("p (kt o) -> p kt o", kt=KT)
                wv = w_views[l]
                src = wts[l][:]
                cw = OUT // nchunk
                for c in chunks if chunks is not None else range(nchunk):
                    chunk_src = bass.AP(
                        tensor=src.tensor, offset=src.offset + c * cw,
                        ap=[[OUT, 128], [128 * OUT, KT], [1, cw]],
                    )
                    nc.gpsimd.dma_start(out=wv[:, :, c * cw : (c + 1) * cw], in_=chunk_src)

            def alloc_xcol(n):
                KT = D0 // 128
                xk = xcolp.tile([128, KT * NB], BF16, tag="xcol", name=f"x{n}")
                xv = xk.rearrange("p (kt b) -> p kt b", kt=KT)
                return xv

            def emit_xchunk(xv, n, c, nchunk):
                KT = D0 // 128
                ck = KT // nchunk
                src = xt[:]
                chunk_src = bass.AP(
                    tensor=src.tensor,
                    offset=src.offset + n * NB + c * ck * 128 * B_LOC,
                    ap=[[B_LOC, 128], [128 * B_LOC, ck], [1, NB]],
                )
                nc.gpsimd.dma_start(out=xv[:, c * ck : (c + 1) * ck, :], in_=chunk_src)

            def load_xcol(n):
                xv = alloc_xcol(n)
                emit_xchunk(xv, n, 0, 1)
                return [xv[:, k, :] for k in range(D0 // 128)]

            # gpsimd queue ordered by first use, x col0 and W1 chunks
            # interleaved so the first o-group can start ~16us in
            load_a(0)
            xv0 = alloc_xcol(0)
            emit_xchunk(xv0, 0, 0, 4)
            emit_xchunk(xv0, 0, 1, 4)
            load_layer(0, nchunk=8, chunks=[0])
            emit_xchunk(xv0, 0, 2, 4)
            load_layer(0, nchunk=8, chunks=[1])
            emit_xchunk(xv0, 0, 3, 4)
            load_layer(0, nchunk=8, chunks=[2, 3])
            load_lb(0)
            nc.gpsimd.dma_start(out=gammas[0][:], in_=gds[0][:])
            load_layer(0, nchunk=8, chunks=[4, 5, 6, 7])
            first_cols = [xv0[:, k, :] for k in range(D0 // 128)]
            load_a(1)
            load_layer(1, nchunk=1)
            load_lb(1)
            nc.gpsimd.dma_start(out=gammas[1][:], in_=gds[1][:])
            load_a(2)
            load_layer(2, nchunk=1)
            load_lb(2)
            nc.gpsimd.dma_start(out=gammas[2][:], in_=gds[2][:])

            def emit_A(l, n, cols, part, k0, k1, KT):
                """LoRA A-side, col-group packed: 4 concurrent partial
                accumulations in one PSUM bank (the four 32-row bands run on
                disjoint PE column groups). Emitted in two ranges: the last
                band group (which depends on the previous layer's final relu)
                is deferred under the first main o-group chain."""
                for k in range(k0, k1):
                    g = k % 4
                    nc.tensor.matmul(
                        part[g * 32 : (g + 1) * 32, :], a_tiles[l][k], cols[k],
                        start=(k < 4), stop=(k >= KT - 4), tile_position=(0, g * 32),
                    )
                    if n == 0 and l == 0 and k % 4 == 3 and k < KT - 1:
                        warmup(10)  # x col-0 chunks arrive ~2.5us apart

            # main fused pipeline: per batch-column, all three layers
            xcols = [first_cols, None, None, None]
            for n in range(NT):
                cols = xcols[n]
                if n + 1 < NT:
                    xcols[n + 1] = load_xcol(n + 1)
                for l, (IN, OUT) in enumerate(DIMS):
                    KT, OT = IN // 128, OUT // 128
                    part = tps.tile([128, NB], F32, tag="tpsum")
                    emit_A(l, n, cols, part, 0, KT - 4, KT)
                    if n == 0 and l == 0:
                        warmup(N_WARM_OG0_WAIT)
                    tw = twp.tile([128, NB], BF16, tag="tw")
                    pt = None

                    nxt = []
                    if n == 0 and l == 0:
                        # small first groups: start once 1MB of W1 has landed
                        ogroups = [(0, 2), (2, 2), (4, 4), (8, 4), (12, 4)]
                    else:
                        ogroups = [(og, min(4, OT - og)) for og in range(0, OT, 4)]
                    for og, gw in ogroups:
                        pss = []
                        for i in range(gw):
                            o = og + i
                            ps = mmps.tile([128, NB], F32, tag="mm")
                            for k in range(KT):
                                nc.tensor.matmul(
                                    ps[:], w_views[l][:, k, o * 128 : (o + 1) * 128],
                                    cols[k], start=(k == 0), stop=False,
                                )
                            if og == 0 and i == 0:
                                # deferred A band group (waits the previous
                                # layer's last relu) + partials copy-out
                                emit_A(l, n, cols, part, KT - 4, KT, KT)
                                pt = twp.tile([128, NB], BF16, tag="tw",
                                              name=f"pt{l}_{n}")
                                nc.vector.tensor_copy(pt[:], part[:])
                            elif og == 0 and i == 1:
                                # combine + gamma-mul hidden under this chain
                                t4 = tps.tile([128, NB], F32, tag="tpsum")
                                nc.tensor.matmul(t4[:], combine_t[:], pt[:],
                                                 start=True, stop=True)
                                nc.vector.tensor_mul(
                                    tw[:], t4[:], gammas[l][:, n * NB : (n + 1) * NB]
                                )
                            pss.append(ps)
                        if l == 2 and og == 0:
                            oc_big = ocp.tile([128, OT * NB], BF16, tag="oc", name=f"oc{n}")
                            ocv = oc_big.rearrange("p (ot b) -> p ot b", ot=OT)
                        for i in range(gw):
                            o = og + i
                            nc.tensor.matmul(
                                pss[i][:],
                                lb_tiles[l][i * 32 : (i + 1) * 32, o * 128 : (o + 1) * 128],
                                tw[i * 32 : (i + 1) * 32, :],
                                start=False, stop=True, tile_position=(i * 32, 0),
                            )
                            if l < 2:
                                pool = h1p if l == 0 else h2p
                                ot = pool.tile([128, NB], BF16, tag=f"h{l + 1}", name=f"h{l}_{n}_{o}")
                                dst = ot[:]
                            else:
                                dst = ocv[:, o, :]
                            if i % 2 == 0:
                                nc.scalar.activation(
                                    dst, pss[i][:], AF.Relu, bias=b_tiles[l][:, o : o + 1]
                                )
                            else:
                                nc.vector.tensor_scalar(
                                    dst, pss[i][:], b_tiles[l][:, o : o + 1], 0.0,
                                    op0=ALU.add, op1=ALU.max,
                                )
                            if l < 2:
                                nxt.append(dst)
                        if l == 2:
                            # last column: two half-transfers so the first can
                            # overlap the remaining relus
                            nhalf = 2 if n == NT - 1 else 1
                            oth = OT // nhalf
                            for hf in range(nhalf):
                                out_dst = bass.AP(
                                    tensor=out_d[:].tensor,
                                    offset=out_d[:].offset + n * NB + hf * oth * 128 * B_LOC,
                                    ap=[[B_LOC, 128], [128 * B_LOC, oth], [1, NB]],
                                )
                                ring = nc.sync if (nhalf == 2 and hf == 1) else nc.gpsimd
                                ring.dma_start(
                                    out=out_dst, in_=ocv[:, hf * oth : (hf + 1) * oth, :]
                                )
                    cols = nxt
                if n == 0:
                    # drain the warmup accumulator mid-run, off the tail
                    wout = const.tile([128, 128], BF16, tag="warm_out", name="warmout")
                    nc.vector.tensor_copy(wout[:], warm_ps[:])
                    nc.scalar.dma_start(out=warm_sink[:], in_=wout[:])
    return nc


_CACHED = {}


def _get_nc():
    if "nc" not in _CACHED:
        nc = bass.Bass()
        _build(nc)
        _patch_bass_json(nc)
        _CACHED["nc"] = nc
    return _CACHED["nc"]


# ---------------------------------------------------------------------------
# Host-side routing: x-independent, tiny -> fold into per-token gamma rows
# ---------------------------------------------------------------------------
def _topk_sparse_softmax_np(logits, k):
    n = logits.shape[-1]
    flat = logits.reshape(-1, n)
    thresh = np.sort(flat, axis=-1)[:, n - k][:, None]
    mask = flat >= thresh
    mx = np.max(flat, axis=-1, keepdims=True)
    e = np.exp(flat - mx) * mask
    return (e / e.sum(-1, keepdims=True)).reshape(logits.shape)


def _routing_host(inputs):
    f = lambda k: np.asarray(inputs[k], np.float64)
    dom_emb, layer_pos = f("dom_emb"), f("layer_pos")
    router_inp = np.concatenate(
        [
            np.broadcast_to(dom_emb[:, None, :], (M, L, H)),
            np.broadcast_to(layer_pos[None, :, :], (M, L, H)),
        ],
        axis=-1,
    )                                                       # [M,L,2H]
    hz = np.maximum(router_inp @ f("Wi1").T + f("bi1"), 0.0)
    zeta_logits = (hz @ f("Wi2").T + f("bi2"))[..., 0]      # [M,L]
    zeta_all = _topk_sparse_softmax_np(zeta_logits, 2)
    ha = np.maximum(router_inp @ f("Wa1").T + f("ba1"), 0.0)
    alpha_logits = ha @ f("Wa2").T + f("ba2")               # [M,L,E]
    alpha_all = _topk_sparse_softmax_np(alpha_logits, 2)
    g = f("gate_logits")
    R = np.log1p(np.exp(-np.abs(g))) + np.maximum(g, 0.0)   # stable softplus
    R = R * f("R_benefit")
    R = R / np.clip(R.sum(-1, keepdims=True), 1e-12, None)
    zeta_agg = R @ zeta_all                                 # [M,L]
    alpha_agg = np.einsum("mn,nle->mle", R, alpha_all)      # [M,L,E]
    return (zeta_agg[:, :, None] * alpha_agg).astype(np.float32)  # [M,L,E]


def kernel(**inputs) -> np.ndarray:
    x = np.asarray(inputs["x"], np.float32)
    ids = np.asarray(inputs["domain_ids"]).astype(np.int64)
    f32 = lambda a: np.ascontiguousarray(np.asarray(a), np.float32)
    bf = lambda a: np.ascontiguousarray(np.asarray(a, np.float32).astype(BF_NP))

    W = [f32(inputs[f"W{i}"]) for i in (1, 2, 3)]
    Bv = [f32(inputs[f"b{i}"]) for i in (1, 2, 3)]
    A = [f32(inputs[f"A{i}"]) for i in (1, 2, 3)]
    Bl = [f32(inputs[f"B{i}"]) for i in (1, 2, 3)]

    gamma = _routing_host(inputs)                           # [M,L,E]
    gtok = gamma[ids]                                       # [B,L,E]

    shared = {"combine": bf(np.tile(np.eye(E * RK, dtype=np.float32), (4, 4)))}
    for l in range(3):
        shared[f"w{l + 1}t"] = bf(W[l].T)
        shared[f"a{l + 1}t"] = bf(A[l].reshape(E * RK, -1).T)
        shared[f"lb{l + 1}"] = bf(np.tile(Bl[l].transpose(0, 2, 1).reshape(E * RK, -1), (4, 1)))
        shared[f"bias{l + 1}"] = Bv[l]

    in_maps = []
    for i in range(N_CORES):
        sl = slice(i * B_LOC, (i + 1) * B_LOC)
        m = dict(shared)
        m["xt"] = bf(x[sl].T)
        for l in range(3):
            g32 = np.repeat(gtok[sl, l, :], RK, axis=1).T   # [32, B_LOC]
            m[f"g{l + 1}"] = bf(np.tile(g32, (4, 1)))       # [128, B_LOC]
        in_maps.append(m)

    nc = _get_nc()
    res = run_bass_kernel_spmd(nc, in_maps, core_ids=list(range(N_CORES)))
    return np.concatenate(
        [np.asarray(res.results[i]["out"]).astype(np.float32).T for i in range(N_CORES)],
        axis=0,
    )
